# revision 38
# baseline (speedup 1.0000x reference)
"""Fused multi-head attention layer for Trainium2, 8-core data-parallel.

Problem: x[8,1024,768] -> qkv proj (w_qkv[2304,768]) -> 12-head attention
(head_dim 64, key-padding mask) -> out proj (w_proj[768,768] + b_proj).

Strategy (v3):
  * Data parallel over batch: core b handles x[b] end to end. No collectives.
  * All matmul operands bf16 (host-converted); PSUM accumulation fp32.
  * The PE is the bottleneck engine (~344K cycles = ~143us at 2.4GHz:
    qkv-proj 110.6K, scores 98K, AV 98K, out-proj 37K). The schedule is
    built to keep the PE instruction stream dense end to end; the ACT
    exp stream (~118us) hides underneath it.
  * Scores are computed TRANSPOSED, S.T[m,l] = K @ Q.T, in HALF-TILES
    [128,512] so each PSUM slot is one bank. Four 1-bank score slots
    rotate; softmax exp (scalar engine, scale+mask-bias folded in)
    evacuates each slot to a bf16 P tile half.
  * The QKV projection runs through the same 4-slot rotation: only pair
    0's Q/K e-tiles run up front; later pairs' e-tiles DRIP through the
    rotation as 6-matmul half-chains (one per score step), so the
    in-order PE stream never head-of-line blocks the exp stream.
  * AV matmuls for pair t-1 interleave with scores/exps of pair t
    (P-tile pool is deep enough to lag a full pair); V projection rides
    in pair 0's window via a dedicated 4-bank PSUM pool that closes
    before the AV accumulators open. The softmax denominator comes free
    from a ones column appended to V (row 64 of the AV accumulator).
  * Output projection is issued as 16 [128,384] half-chains the moment
    the score pool closes: k=0..4 partials overlap pair 5's epilogue
    (normalization chain on DVE/Pool), and the PE stays ramped through
    the boundary (no p-state reset).
  * PSUM bank budget over time: scores(4) -> +V(4)=8 -> V->AV swap(4+4)
    -> close AV+scores after the last epilogue -> outproj(8). Max 8.
  * Startup: the DMA fabric is one serial transfer pipe plus one serial
    trigger engine, so all input DMAs go down a single HWDGE queue in
    strict priority order (Q0K0 | xT chunks | Q1K1 | V | drip pairs |
    w2/bias); w_qkv is host-packed [Q0K0|Q1K1|..|Q5K5|V] with the
    on-chip e-axis permuted to match, one contiguous DMA per group.
    Junk warmup matmuls ramp the PE p-state while DMAs fly.
  * Final projection bias rides a K=1 ones(x)bias outer-product matmul
    into the same PSUM accumulators; output is staged bf16 and DMA'd
    per l-tile.
"""

import os
import sys

import numpy as np

sys.path.insert(0, "/opt/trn_rl_repo")

B, L, D, H, HD = 8, 1024, 768, 12, 64
E = 3 * D
SCALE = HD ** -0.5
P = 128
KC = D // P          # 6 contraction chunks of 128 over d
LT = L // P          # 8 l/m partition tiles
NP = H // 2          # 6 head pairs
NCORES = 8
NEG = -30000.0       # mask bias; exp(NEG + s) == 0 in fp32
HC = 512             # score half-tile width (one PSUM bank of fp32)

# host w1 packing order: (Qp, Kp) pairs then V. On-chip, w1T_sb's e-axis
# is PERMUTED to match ([Q0 K0 Q1 K1 .. Q5 K5 V]) so each pair is a
# single contiguous DMA.
WGROUPS = [[(p * P, P), (D + p * P, P)] for p in range(NP)] + [[(2 * D, D)]]


def _etcol(et):
    # column base of e-tile `et` in the interleaved w1T_sb layout
    return 2 * (et % KC) * P + (P if et >= KC else 0)

_cached = {}


def _build_program(reps=1, phases='ABC', loop_n=0):
    import concourse.tile as tile
    from concourse import bacc, mybir

    f32 = mybir.dt.float32
    bf16 = mybir.dt.bfloat16
    AF = mybir.ActivationFunctionType

    nc = bacc.Bacc(trn_type="TRN2", target_bir_lowering=False, debug=False)

    # host pre-swizzled layouts: partition-major, contiguous per partition
    xT_d = nc.declare_dram_parameter("xT", [P, KC * L], bf16, isOutput=False)
    w1e_d = nc.declare_dram_parameter("w1e", [P, KC * E], bf16, isOutput=False)
    w2T_d = nc.declare_dram_parameter("w2T", [P, KC * D], bf16, isOutput=False)
    b2r_d = nc.declare_dram_parameter("b2r", [1, D], bf16, isOutput=False)
    mbias_d = nc.declare_dram_parameter("mbias", [P, LT], f32, isOutput=False)
    out_d = nc.declare_dram_parameter("out", [P, LT * D], bf16, isOutput=True)

    with tile.TileContext(nc) as tc:
      from contextlib import ExitStack, nullcontext

      with tc.For_i(0, loop_n, 1) if loop_n else nullcontext():
       for _rep in range(reps):
        with ExitStack() as ctx:
            persist = ctx.enter_context(tc.tile_pool(name="persist", bufs=1))
            # qkvT for Q and K: e-tiles 0..5 = Q pairs, 6..11 = K pairs
            qkT_sb = persist.tile([P, 2 * KC, L], bf16)
            # V with a ones column per head: [l-tile, head, 65]
            V_sb = persist.tile([P, LT, H * (HD + 1)], bf16)
            V_v = V_sb[:].rearrange("p l (h c) -> p l h c", c=HD + 1)
            OT_sb = persist.tile([P, KC, L], bf16)      # O.T, heads stacked
            bias_sb = persist.tile([P, LT], f32)        # mask bias per key pos
            w2Tb_sb = persist.tile([P, KC, D], bf16)
            b2r_sb = persist.tile([1, D], bf16)         # proj bias row
            onesr_sb = persist.tile([1, P], bf16)       # ones row (bias mm)

            # ---------------- input DMAs, priority-ordered ----------------
            # ones column of V' via a single DVE memset
            nc.vector.memset(V_v[:, :, :, HD], 1.0)
            nc.vector.memset(onesr_sb[:], 1.0)

            pA = ctx.enter_context(tc.tile_pool(name="phA", bufs=1))
            xT_sb = pA.tile([P, KC, L], bf16)
            w1T_sb = pA.tile([P, KC, E], bf16)
            junk_sb = pA.tile([P, HC], bf16)
            nc.gpsimd.memset(junk_sb[:], 0.0)
            xT_r = xT_d.ap().rearrange("p (k l) -> p k l", l=L)

            def w1grp(eng, p, off):
                # one (Qp,Kp) pair = 256 contiguous e-cols in the
                # interleaved layout; V = the final 768
                ew = 2 * P if p < NP else D
                eng.dma_start(
                    out=w1T_sb[:, :, 2 * P * p : 2 * P * p + ew],
                    in_=w1e_d[:, off : off + KC * ew].rearrange(
                        "p (k e) -> p k e", e=ew
                    ),
                )
                return off + KC * ew

            # the DMA fabric is one serial transfer pipe + one serial
            # trigger engine (~0.6us/DMA), so a single queue in strict
            # priority order gives exactly the transfer order we want:
            # Q0K0, the six xT chunks, Q1K1, V, the drip pairs, then
            # phase-C weights.
            w1grp(nc.sync, 0, 0)
            for k in range(KC):
                nc.sync.dma_start(out=xT_sb[:, k, :], in_=xT_r[:, k, :])
            w1grp(nc.sync, 1, KC * 2 * P)
            w1grp(nc.sync, NP, KC * 2 * P * NP)
            off = KC * 2 * P * 2
            for p in range(2, NP):
                off = w1grp(nc.sync, p, off)
            nc.sync.dma_start(
                out=w2Tb_sb[:, :, :],
                in_=w2T_d.ap().rearrange("p (k f) -> p k f", f=D),
            )
            nc.sync.dma_start(out=b2r_sb[:], in_=b2r_d.ap())
            if "M" in phases:
                nc.scalar.dma_start(out=bias_sb[:], in_=mbias_d.ap())
            if "D" in phases:
                continue

            ptp = ctx.enter_context(tc.tile_pool(name="pt", bufs=10))
            pn = ctx.enter_context(tc.tile_pool(name="norm", bufs=1))

            # ---- PSUM pools with staggered lifetimes (8-bank budget) ----
            psS_cm = tc.tile_pool(name="psS", bufs=4, space="PSUM")
            psS = psS_cm.__enter__()

            def qk_half(et, c):
                # half e-tile of the QKV projection through a score slot:
                # qkT[e, l-half] = w1.T.T @ xT
                ec = _etcol(et)
                ps = psS.tile([P, HC], f32, tag="s")
                for k in range(KC):
                    nc.tensor.matmul(
                        ps[:],
                        lhsT=w1T_sb[:, k, ec : ec + P],
                        rhs=xT_sb[:, k, c * HC : (c + 1) * HC],
                        start=(k == 0),
                        stop=(k == KC - 1),
                    )
                nc.vector.tensor_copy(
                    qkT_sb[:, et, c * HC : (c + 1) * HC], ps[:]
                )

            def s_half(t, j, hh, c):
                # S.T[m-tile j, l-half c] for head 2t+hh; operands sit on
                # partitions [64*hh : 64*hh+64] -> PE row tile 0 or 64
                kb = 64 * hh
                ps = psS.tile([P, HC], f32, tag="s")
                nc.tensor.matmul(
                    ps[:],
                    lhsT=qkT_sb[kb : kb + 64, KC + t, j * P : (j + 1) * P],
                    rhs=qkT_sb[kb : kb + 64, t, c * HC : (c + 1) * HC],
                    start=True,
                    stop=True,
                )
                return ps

            def s_burst(t, j):
                return [
                    (hh, c, s_half(t, j, hh, c))
                    for hh in (0, 1)
                    for c in (0, 1)
                ]

            def exps(t, j, burst):
                pt0 = ptp.tile([P, L], bf16, tag="pt0")
                pt1 = ptp.tile([P, L], bf16, tag="pt1")
                pts = (pt0, pt1)
                for hh, c, ps in burst:
                    if "M" in phases:
                        nc.scalar.activation(
                            pts[hh][:, c * HC : (c + 1) * HC],
                            ps[:],
                            AF.Exp,
                            bias=bias_sb[:, j : j + 1],
                            scale=SCALE,
                        )
                    else:
                        nc.scalar.activation(
                            pts[hh][:, c * HC : (c + 1) * HC],
                            ps[:],
                            AF.Exp,
                            bias=0.0,
                            scale=SCALE,
                        )
                return pts

            def v_chain(psp, i):
                # V[l-tile i, dv] = x @ w1_v.T  (dv in [1536, 2304))
                ps = psp.tile([P, D], f32, tag="v")
                for k in range(KC):
                    for c0, cw in ((0, HC), (HC, D - HC)):
                        nc.tensor.matmul(
                            ps[:, c0 : c0 + cw],
                            lhsT=xT_sb[:, k, i * P : (i + 1) * P],
                            rhs=w1T_sb[:, k, 2 * D + c0 : 2 * D + c0 + cw],
                            start=(k == 0),
                            stop=(k == KC - 1),
                        )
                for c in range(2):
                    dst = V_v[:, i, 6 * c : 6 * (c + 1), 0:HD]
                    src = ps[:, c * 384 : (c + 1) * 384].rearrange(
                        "p (h q) -> p h q", q=HD
                    )
                    nc.vector.tensor_copy(dst, src)

            def avs(t, j, pts, oA, oB):
                for hh, ot in ((0, oA), (1, oB)):
                    h = 2 * t + hh
                    for c in range(2):
                        nc.tensor.matmul(
                            ot[0:65, c * HC : (c + 1) * HC],
                            lhsT=V_v[:, j, h, :],
                            rhs=pts[hh][:, c * HC : (c + 1) * HC],
                            start=(j == 0),
                            stop=(j == LT - 1),
                        )

            def epilogue_fast(t, oA, oB):
                # pair-5 variant: the out-projection's k=5 matmuls gate on
                # this chain, so recips run early and the broadcast/mul/
                # row-swap go per l-half to unlock chains progressively
                osA = pn.tile([65, L], f32, tag="osA")
                osB = pn.tile([65, L], f32, tag="osB")
                den0 = pn.tile([1, 2, L], f32, tag="den0")
                denr = pn.tile([1, 2, L], f32, tag="denr")
                rep = pn.tile([64, 2, L], f32, tag="rep")
                btmp = pn.tile([64, L], bf16, tag="btmp")
                nc.vector.tensor_copy(osB[:], oB[0:65, :])
                nc.sync.dma_start(out=den0[0:1, 1, :], in_=osB[64:65, :])
                nc.vector.tensor_copy(osA[:], oA[0:65, :])
                nc.sync.dma_start(out=den0[0:1, 0, :], in_=osA[64:65, :])
                nc.vector.reciprocal_approx_fast(
                    denr[0:1, 1, :], den0[0:1, 1, :]
                )
                nc.vector.reciprocal_approx_fast(
                    denr[0:1, 0, :], den0[0:1, 0, :]
                )
                for lh in range(2):
                    s = slice(lh * HC, (lh + 1) * HC)
                    nc.gpsimd.partition_broadcast(
                        rep[0:64, 1, s], denr[0:1, 1, s], channels=64
                    )
                    nc.vector.tensor_mul(
                        btmp[0:64, s], osB[0:64, s], rep[0:64, 1, s]
                    )
                    nc.sync.dma_start(
                        out=OT_sb[64:128, t, s], in_=btmp[0:64, s]
                    )
                    nc.gpsimd.partition_broadcast(
                        rep[0:64, 0, s], denr[0:1, 0, s], channels=64
                    )
                    nc.vector.tensor_mul(
                        OT_sb[0:64, t, s], osA[0:64, s], rep[0:64, 0, s]
                    )

            def epilogue(t, oA, oB):
                # evacuate the AV accumulators (frees their PSUM slots,
                # head A first: the next pair's first AV gates on the oA
                # slot), then normalize by the ones-row denominator off
                # the critical path (DVE recip + Pool broadcast + DVE mul)
                osA = pn.tile([65, L], f32, tag="osA")
                osB = pn.tile([65, L], f32, tag="osB")
                den0 = pn.tile([1, 2, L], f32, tag="den0")
                denr = pn.tile([1, 2, L], f32, tag="denr")
                rep = pn.tile([64, 2, L], f32, tag="rep")
                btmp = pn.tile([64, L], bf16, tag="btmp")
                nc.vector.tensor_copy(osA[:], oA[0:65, :])
                nc.sync.dma_start(out=den0[0:1, 0, :], in_=osA[64:65, :])
                nc.vector.tensor_copy(osB[:], oB[0:65, :])
                nc.sync.dma_start(out=den0[0:1, 1, :], in_=osB[64:65, :])
                nc.vector.reciprocal_approx_fast(
                    denr[0:1, 1, :], den0[0:1, 1, :]
                )
                nc.gpsimd.partition_broadcast(
                    rep[0:64, 1, :], denr[0:1, 1, :], channels=64
                )
                nc.vector.tensor_mul(
                    btmp[0:64, :], osB[0:64, :], rep[0:64, 1, :]
                )
                nc.sync.dma_start(out=OT_sb[64:128, t, :], in_=btmp[0:64, :])
                nc.vector.reciprocal_approx_fast(
                    denr[0:1, 0, :], den0[0:1, 0, :]
                )
                nc.gpsimd.partition_broadcast(
                    rep[0:64, 0, :], denr[0:1, 0, :], channels=64
                )
                nc.vector.tensor_mul(
                    OT_sb[0:64, t, :], osA[0:64, :], rep[0:64, 0, :]
                )

            # ---------------- phase B: fused pair pipeline ----------------
            pts_d = {}
            psV_cm = tc.tile_pool(name="psV", bufs=2, space="PSUM")
            psV = psV_cm.__enter__()
            psO = None
            oAB = {}

            # p-state warmup: junk matmuls ramp the PE to max clock while
            # the first xT/w1 DMA chunks are still in flight
            for _ in range(6):
                psj = psS.tile([P, HC], f32, tag="s", name="psj")
                nc.tensor.matmul(
                    psj[:], lhsT=junk_sb[:, 0:P], rhs=junk_sb[:],
                    start=True, stop=True,
                )
            # first Q0/K0 chains k-interleaved so each arriving xT chunk
            # feeds two matmuls (DMA-paced startup)
            for c in (0, 1):
                psq = psS.tile([P, HC], f32, tag="s", name=f"psq{c}")
                psk = psS.tile([P, HC], f32, tag="s", name=f"psk{c}")
                for k in range(KC):
                    for ps, et in ((psq, 0), (psk, KC)):
                        nc.tensor.matmul(
                            ps[:],
                            lhsT=w1T_sb[:, k, _etcol(et) : _etcol(et) + P],
                            rhs=xT_sb[:, k, c * HC : (c + 1) * HC],
                            start=(k == 0),
                            stop=(k == KC - 1),
                        )
                for ps, et in ((psq, 0), (psk, KC)):
                    nc.vector.tensor_copy(
                        qkT_sb[:, et, c * HC : (c + 1) * HC], ps[:]
                    )

            for t in range(NP):
                if t == 1:
                    # V pool closed at end of t=0; AV accumulators open
                    psO_cm = tc.tile_pool(name="psO", bufs=1, space="PSUM")
                    psO = psO_cm.__enter__()
                if t >= 1:
                    oA_t = psO.tile([P, L], f32, tag="oA", name=f"oA{t - 1}")
                    oB_t = psO.tile([P, L], f32, tag="oB", name=f"oB{t - 1}")
                    oAB[t - 1] = (oA_t, oB_t)
                burst = s_burst(t, 0)
                # drip schedule for pair t+1's Q/K half-chains
                drip = (
                    [(t + 1, 0), (KC + t + 1, 0), (t + 1, 1), (KC + t + 1, 1)]
                    if t + 1 < NP
                    else []
                )
                dripat = {1: 0, 2: 1, 4: 2, 5: 3}
                for j in range(LT):
                    pts_d[(t, j)] = exps(t, j, burst)
                    if j + 1 < LT:
                        burst = s_burst(t, j + 1)
                    if t >= 1:
                        avs(t - 1, j, pts_d.pop((t - 1, j)), *oAB[t - 1])
                    if t == 0:
                        v_chain(psV, j)
                    if drip and j in dripat:
                        qk_half(*drip[dripat[j]])
                if t == 0:
                    psV_cm.__exit__(None, None, None)
                if t >= 1:
                    epilogue(t - 1, *oAB.pop(t - 1))
            # last pair's AVs + epilogue
            oA = psO.tile([P, L], f32, tag="oA")
            oB = psO.tile([P, L], f32, tag="oB")
            for j in range(LT):
                avs(NP - 1, j, pts_d.pop((NP - 1, j)), oA, oB)
            epilogue_fast(NP - 1, oA, oB)
            # all psS/psO uses are emitted; release LIFO so the
            # out-projection pool can reuse their banks (the scheduler's
            # bank deps let k<=4 partials start right after the oA/oB
            # evacuation, overlapping the rest of the epilogue chain)
            psO_cm.__exit__(None, None, None)
            psS_cm.__exit__(None, None, None)

            if "C" not in phases:
                continue
            # ---------------- phase C: output projection ----------------
            psC_cm = tc.tile_pool(name="psC", bufs=8, space="PSUM")
            psC = psC_cm.__enter__()
            pC = ctx.enter_context(tc.tile_pool(name="pC", bufs=8))
            out_r = out_d.ap().rearrange("p (i f) -> p i f", f=D)
            HW = 384
            for i in range(LT):
                obi = pC.tile([P, D], bf16, tag="ob", name=f"ob{i}")
                for hf in range(2):
                    ps = psC.tile([P, HW], f32, tag="prj")
                    for k in range(KC):
                        nc.tensor.matmul(
                            ps[:],
                            lhsT=OT_sb[:, k, i * P : (i + 1) * P],
                            rhs=w2Tb_sb[:, k, hf * HW : (hf + 1) * HW],
                            start=(k == 0),
                            stop=False,
                        )
                    # bias add as a K=1 ones (x) bias outer product into
                    # the same accumulator, so the evacuation is a plain
                    # copy that can alternate between the idle ACT and DVE
                    nc.tensor.matmul(
                        ps[:],
                        lhsT=onesr_sb[0:1, :],
                        rhs=b2r_sb[0:1, hf * HW : (hf + 1) * HW],
                        start=False,
                        stop=True,
                    )
                    if hf == 0:
                        nc.scalar.copy(obi[:, 0:HW], ps[:])
                    else:
                        nc.vector.tensor_copy(obi[:, HW:D], ps[:])
                # one bf16 output DMA per l-tile (halves trigger count
                # and pipe bytes on the serial DMA fabric)
                nc.sync.dma_start(out=out_r[:, i, :], in_=obi[:])
            psC_cm.__exit__(None, None, None)

    nc.compile()
    return nc


def _get_program(reps=1, phases="ABC", loop_n=0):
    key = f"nc{reps}{phases}L{loop_n}"
    if key not in _cached:
        _cached[key] = _build_program(reps, phases, loop_n)
    return _cached[key]


def _prep_inputs(x, attn_mask, w_qkv, w_proj, b_proj):
    import ml_dtypes

    BF16 = np.dtype(ml_dtypes.bfloat16)
    x = np.asarray(x, dtype=np.float32)
    attn_mask = np.asarray(attn_mask)
    w1T = np.ascontiguousarray(np.asarray(w_qkv, np.float32).T)        # [768, 2304]
    w2T = np.ascontiguousarray(np.asarray(w_proj, np.float32).T)       # [768, 768]

    def swz(a, inner):
        # [KC*P, inner] -> [P, KC*inner], partition-major contiguous
        return np.ascontiguousarray(
            a.reshape(KC, P, inner).transpose(1, 0, 2).reshape(P, KC * inner)
        )

    w1k = swz(w1T, E).reshape(P, KC, E)
    # group-major packing matching WGROUPS: each group's e-ranges are
    # interleaved along e FIRST (so the device's k-major [P, KC*ew]
    # read sees [Qp|Kp] contiguously per k), then groups concatenate
    w1e = np.concatenate(
        [
            np.concatenate(
                [w1k[:, :, e0 : e0 + ew] for e0, ew in grp], axis=2
            ).reshape(P, -1)
            for grp in WGROUPS
        ],
        axis=1,
    ).astype(BF16)
    w2Ts = swz(w2T, D).astype(BF16)
    b2r = np.asarray(b_proj, np.float32).reshape(1, D).astype(BF16)
    in_maps = []
    for b in range(B):
        xT = swz(np.ascontiguousarray(x[b].T), L).astype(BF16)          # [128, 6144]
        mb = NEG * (1 - attn_mask[b].astype(np.float32))                # [1024]
        mbs = np.ascontiguousarray(mb.reshape(LT, P).T.astype(np.float32))
        in_maps.append(
            {
                "xT": xT,
                "w1e": w1e,
                "w2T": w2Ts,
                "b2r": b2r,
                "mbias": mbs,
            }
        )
    return in_maps


def run(x, attn_mask, w_qkv, w_proj, b_proj, trace=False, **spmd_kwargs):
    from concourse.bass_utils import run_bass_kernel_spmd

    # trivial (all-ones) key-padding mask -> bias-free exp path; any other
    # mask falls back to the per-partition additive-bias program
    trivial = bool((np.asarray(attn_mask) == 1).all())
    nc = _get_program(phases="ABC" if trivial else "ABCM")
    in_maps = _prep_inputs(x, attn_mask, w_qkv, w_proj, b_proj)
    res = run_bass_kernel_spmd(
        nc, in_maps, list(range(NCORES)), trace=trace, **spmd_kwargs
    )
    outs = []
    for b in range(B):
        o = np.asarray(res.results[b]["out"])                       # [128, 8*768]
        outs.append(
            o.reshape(P, LT, D).transpose(1, 0, 2).reshape(L, D)
        )
    return np.stack(outs, axis=0).astype(np.float32), res


def kernel(x, attn_mask, w_qkv, w_proj, b_proj):
    out, _ = run(x, attn_mask, w_qkv, w_proj, b_proj)
    return out
